# revision 12
# baseline (speedup 1.0000x reference)
"""Trainium2 Bass kernel for nn_CrossAttention_82429012345074.

8-head self-attention, B=2, N=4096, d_model=512, 8 heads x 64 dim.

Sharding: one head per NeuronCore (8 heads / 8 cores) — tensor parallel:
to_q/k/v column-parallel (each core gets its head's 64 rows of Wq/Wk/Wv),
to_out row-parallel (each core gets its head's 64 columns of Wo and emits a
partial [tok, 512] output). The unshard step sums the 8 partials + bias on
host.

Per-core device kernel (matmuls bf16, fp32 accumulation):
  xT = x pre-transposed on host            # [512f, 8192t] in 4 chunks
  qT/kT = W_dup @ xT                       # [128, 8192] (dh duplicated hi/lo)
  v     = xT.T @ Wv (natural layout)       # [8192, 64] + ones column
  per (batch, 512-query group), chunks of 2 key blocks:
     sA/sB = concurrent row-tile matmul pair (tile_position (0,0)/(64,0))
     p = exp(s * scale): alternating engines —
         ScalarE: ACT exp activation (PSUM->SBUF bf16)
         VectorE: Schraudolph bit-trick exp: int16(s*A+B) bits == bf16 exp
     o[65, q] += [v|1].T @ p               # accumulate; row 64 = denom
     epilogue (denT transpose, recip, out-proj, scaled store) interleaved
     into the next group's chunk loop.
"""

import sys

sys.path.insert(0, "/opt/trn_rl_repo")

import numpy as np
import ml_dtypes

B, N, D, H, DH = 2, 4096, 512, 8, 64
TOK = B * N            # 8192
NQ = 512               # query-group width
NCH = D // 128         # 4 feature chunks of x
NJB = N // 128         # 32 key blocks per batch
NTB = TOK // 128       # 64 token blocks
CH = 2                 # key blocks per exp chunk
NCK = NJB // CH        # 16 chunks per group
SCALE = DH ** -0.5
# Schraudolph exp: fp8e4m3 bits of exp(s*SCALE) ~= int8(s*EXPA + EXPB)
EXPA = float(SCALE * 8 / np.log(2.0))
EXPB = float(7 * 8 - 0.45)
DVE_CHUNKS = frozenset(range(1, 16, 2))  # exp chunks on VectorE (odd ci)
VW = 80                # vP row pitch (fp8, %16==0 for DoubleRow ldweights)
PVD = 2                # PV deferral depth in chunks


def build_bass():
    from contextlib import ExitStack

    import concourse.bass as bass
    import concourse.mybir as mybir
    import concourse.tile as tile
    from concourse import bacc

    f32 = mybir.dt.float32
    bf16 = mybir.dt.bfloat16
    fp8 = mybir.dt.float8e4
    i8 = mybir.dt.int8
    EXP = mybir.ActivationFunctionType.Exp
    MUL = mybir.AluOpType.mult
    ADD = mybir.AluOpType.add
    DR = mybir.MatmulPerfMode.DoubleRow

    nc = bacc.Bacc("TRN2", target_bir_lowering=False, num_devices=8)
    x_d = nc.dram_tensor("x", [NCH, 128, TOK], bf16, kind="ExternalInput")
    wq_d = nc.dram_tensor("wq", [128, NCH, 128], bf16, kind="ExternalInput")
    wk_d = nc.dram_tensor("wk", [128, NCH, 128], bf16, kind="ExternalInput")
    wv_d = nc.dram_tensor("wv", [128, NCH, DH], bf16, kind="ExternalInput")
    wo_d = nc.dram_tensor("wo", [DH, D], bf16, kind="ExternalInput")
    out_d = nc.dram_tensor("out", [TOK, D], f32, kind="ExternalOutput")

    with tile.TileContext(nc) as tc, ExitStack() as ctx:
        const = ctx.enter_context(tc.tile_pool(name="const", bufs=1))
        sb_p = ctx.enter_context(tc.tile_pool(name="sb_p", bufs=4))
        sb_io = ctx.enter_context(tc.tile_pool(name="sb_io", bufs=3))
        ps_s = ctx.enter_context(tc.tile_pool(name="ps_s", bufs=2, space="PSUM"))
        ps_sm = ctx.enter_context(tc.tile_pool(name="ps_sm", bufs=2, space="PSUM"))

        # Long-lived SBUF tensors
        xT = const.tile([128, NCH, TOK], bf16, name="xT")
        qT = const.tile([128, TOK], bf16, name="qT")       # rows 64:128 dup of 0:64
        kT = const.tile([128, TOK], bf16, name="kT")
        vP = const.tile([128, NTB, VW], fp8, name="vP")
        oN = const.tile([DH + 1, TOK], bf16, name="oN")    # row DH = softmax denom
        wq = const.tile([128, NCH, 128], bf16, name="wq")
        wk = const.tile([128, NCH, 128], bf16, name="wk")
        wv = const.tile([128, NCH, DH], bf16, name="wv")
        wo = const.tile([64, D], bf16, name="wo")

        nc.vector.memset(vP[:, :, DH : DH + 1], 1.0)
        ones128 = const.tile([128, 1], bf16, name="ones128")
        nc.vector.memset(ones128, 1.0)

        nc.sync.dma_start(out=wq, in_=wq_d[:])
        nc.sync.dma_start(out=wk, in_=wk_d[:])
        nc.sync.dma_start(out=wv, in_=wv_d[:])
        nc.sync.dma_start(out=wo, in_=wo_d[:])

        # Phase 0: x slab-ordered so early projections can start while the
        # rest of x streams in.
        SLAB = 1024
        # first slab split 4-ways across more DMA queues for a fast start
        for c in range(NCH):
            for h in range(4):
                h0 = h * (SLAB // 4)
                nc.sync.dma_start(
                    out=xT[:, c, h0 : h0 + SLAB // 4],
                    in_=x_d[c, :, h0 : h0 + SLAB // 4],
                )
        for tt in range(1, TOK // SLAB):
            t0 = tt * SLAB
            for c in range(NCH):
                nc.sync.dma_start(
                    out=xT[:, c, t0 : t0 + SLAB],
                    in_=x_d[c, :, t0 : t0 + SLAB],
                )

        # Phase 1: projections
        for g in range(TOK // NQ):
            t0 = g * NQ
            qp = ps_s.tile([128, NQ], f32, tag="s", name="qp")
            for c in range(NCH):
                nc.tensor.matmul(
                    qp, lhsT=wq[:, c, :], rhs=xT[:, c, t0 : t0 + NQ],
                    start=(c == 0), stop=(c == NCH - 1),
                )
            nc.scalar.copy(out=qT[:, t0 : t0 + NQ], in_=qp)

            kp = ps_s.tile([128, NQ], f32, tag="s", name="kp")
            for c in range(NCH):
                nc.tensor.matmul(
                    kp, lhsT=wk[:, c, :], rhs=xT[:, c, t0 : t0 + NQ],
                    start=(c == 0), stop=(c == NCH - 1),
                )
            nc.scalar.copy(out=kT[:, t0 : t0 + NQ], in_=kp)

            for t in range(NQ // 128):
                tb = g * (NQ // 128) + t
                vp = ps_sm.tile([128, DH], f32, tag="o", name="vp")
                for c in range(NCH):
                    nc.tensor.matmul(
                        vp, lhsT=xT[:, c, tb * 128 : tb * 128 + 128], rhs=wv[:, c, :],
                        start=(c == 0), stop=(c == NCH - 1),
                    )
                nc.vector.tensor_copy(out=vP[:, tb, 0:DH], in_=vp)

        # Phase 2+3: attention + output projection.
        def emit_denT(pq0):
            denT = ps_sm.tile([128, 4], f32, tag="o", name="denT")
            for t in range(NQ // 128):
                nc.tensor.matmul(
                    denT[:, t : t + 1],
                    lhsT=oN[DH : DH + 1, pq0 + t * 128 : pq0 + (t + 1) * 128],
                    rhs=ones128[DH : DH + 1, :],
                    start=True, stop=True,
                )
            recT = sb_io.tile([128, 4], f32, tag="rec", bufs=2, name="recT")
            nc.vector.reciprocal(recT, denT)
            return recT

        fp_queue = {}

        def emit_fp(pq0, t):
            tt0 = pq0 + t * 128
            fp = ps_sm.tile([128, D], f32, tag="o", name="fp")
            nc.tensor.matmul(
                fp, lhsT=oN[0:DH, tt0 : tt0 + 128], rhs=wo, start=True, stop=True
            )
            fp_queue[t] = fp

        def emit_ob(pq0, t, recT, on_act=False):
            tt0 = pq0 + t * 128
            ob = sb_io.tile([128, D], f32, tag="ob", name="ob")
            if on_act:
                nc.scalar.mul(ob, fp_queue.pop(t), recT[:, t : t + 1])
            else:
                nc.vector.tensor_scalar_mul(
                    ob, in0=fp_queue.pop(t), scalar1=recT[:, t : t + 1]
                )
            nc.sync.dma_start(out=out_d[tt0 : tt0 + 128, :], in_=ob)

        pending = None  # q0 of the previous group
        recT = None
        for b in range(B):
            for qg in range(N // NQ):
                q0 = b * N + qg * NQ
                o = ps_sm.tile([DH + 1, NQ], f32, tag="oo", name="o")
                # PV deferred PVD chunks so each exp overlaps ~2 chunks of
                # PE work before its PV drains it.
                pv_queue = []  # (p tile, chunk index) awaiting emission

                def flush_pv(pv):
                    p, pci = pv
                    jbg = b * NJB + pci * CH
                    nc.tensor.matmul(
                        o, lhsT=vP[:, jbg : jbg + CH, 0 : DH + 1],
                        rhs=p[:, 0:CH, :],
                        start=(pci == 0), stop=(pci == NCK - 1),
                        perf_mode=DR,
                    )

                for ci in range(NCK):
                    jb = ci * CH
                    s = ps_s.tile([128, CH, NQ], f32, tag="s", name="s")
                    jA = b * N + jb * 128
                    jB = b * N + (jb + 1) * 128
                    nc.tensor.matmul(
                        s[:, 0, :], lhsT=kT[0:64, jA : jA + 128],
                        rhs=qT[0:64, q0 : q0 + NQ], start=True, stop=True,
                    )
                    nc.tensor.matmul(
                        s[:, 1, :], lhsT=kT[64:128, jB : jB + 128],
                        rhs=qT[64:128, q0 : q0 + NQ], start=True, stop=True,
                    )
                    p = sb_p.tile([128, CH, NQ], fp8, name="p")
                    if ci in DVE_CHUNKS:
                        nc.vector.tensor_scalar(
                            out=p.bitcast(i8), in0=s[:, 0:CH, :],
                            scalar1=EXPA, scalar2=EXPB, op0=MUL, op1=ADD,
                        )
                    else:
                        nc.scalar.activation(
                            out=p[:, 0:CH, :], in_=s[:, 0:CH, :], func=EXP, scale=SCALE
                        )
                    if len(pv_queue) >= PVD:
                        flush_pv(pv_queue.pop(0))
                    pv_queue.append((p, ci))
                    # previous group's epilogue, spread across this loop
                    if pending is not None:
                        if ci == 1:
                            recT = emit_denT(pending)
                        elif ci in (3, 6, 9, 12):
                            emit_fp(pending, (ci - 3) // 3)
                        elif ci in (4, 7, 10, 13):
                            t = (ci - 4) // 3
                            emit_ob(pending, t, recT, on_act=(t < 3))
                for pv in pv_queue:
                    flush_pv(pv)

                nc.vector.tensor_copy(out=oN[:, q0 : q0 + NQ], in_=o)
                pending = q0

        recT = emit_denT(pending)
        for t in range(NQ // 128):
            emit_fp(pending, t)
            emit_ob(pending, t, recT)

    nc.compile()
    return nc


def make_in_maps(x, Wq, Wk, Wv, Wo):
    bf16 = ml_dtypes.bfloat16
    x_bf = np.ascontiguousarray(
        x.reshape(TOK, D).T.reshape(NCH, 128, TOK)
    ).astype(bf16)
    in_maps = []
    for h in range(H):
        sl = slice(h * DH, (h + 1) * DH)
        # [512, 64] -> [NCH, 128, 64] -> dup dh -> [128, NCH, 128]
        wqc = Wq[sl, :].T.reshape(NCH, 128, DH)
        wkc = Wk[sl, :].T.reshape(NCH, 128, DH)
        wvc = Wv[sl, :].T.reshape(NCH, 128, DH)
        in_maps.append(
            {
                "x": x_bf,
                "wq": np.ascontiguousarray(
                    np.concatenate([wqc, wqc], axis=2).transpose(1, 0, 2)
                ).astype(bf16),
                "wk": np.ascontiguousarray(
                    np.concatenate([wkc, wkc], axis=2).transpose(1, 0, 2)
                ).astype(bf16),
                "wv": np.ascontiguousarray(wvc.transpose(1, 0, 2)).astype(bf16),
                "wo": np.ascontiguousarray(Wo[:, sl].T).astype(bf16),
            }
        )
    return in_maps


def _install_ntff_shim():
    """The axon boot skips registering the NTFF profile hook when the image's
    antenv lacks axon_hooks; register an equivalent shim so trace=True works."""
    import types

    if "antenv.axon_hooks" in sys.modules:
        return
    try:
        from trn_agent_boot.trn_boot import _ntff_profile_via_ctypes

        hook = _ntff_profile_via_ctypes("/opt/axon/libaxon_pjrt.so")
    except Exception:
        hook = None
    mod = types.ModuleType("antenv.axon_hooks")
    mod.get_axon_ntff_profile_hook = lambda: hook
    sys.modules["antenv.axon_hooks"] = mod


def run(x, Wq, Wk, Wv, Wo, bo, trace=False):
    from concourse.bass_utils import run_bass_kernel_spmd

    if trace:
        _install_ntff_shim()

    nc = build_bass()
    in_maps = make_in_maps(x, Wq, Wk, Wv, Wo)
    res = run_bass_kernel_spmd(nc, in_maps, core_ids=list(range(H)), trace=trace)
    acc = np.zeros((TOK, D), dtype=np.float32)
    for r in res.results:
        acc += r["out"]
    acc += np.asarray(bo, dtype=np.float32)[None, :]
    return acc.reshape(B, N, D), res


def kernel(x, Wq, Wk, Wv, Wo, bo):
    out, _ = run(
        np.asarray(x, dtype=np.float32),
        np.asarray(Wq, dtype=np.float32),
        np.asarray(Wk, dtype=np.float32),
        np.asarray(Wv, dtype=np.float32),
        np.asarray(Wo, dtype=np.float32),
        np.asarray(bo, dtype=np.float32),
    )
    return out


if __name__ == "__main__":
    import reference

    inputs = reference.setup_inputs()
    inputs = {k: np.asarray(v) for k, v in inputs.items()}
    expected = np.asarray(reference.reference(**inputs))
    out = kernel(**inputs)
    rel = np.linalg.norm(out - expected) / np.linalg.norm(expected)
    print(f"Relative error: {rel:.3e}")


# revision 13
# speedup vs baseline: 1.1072x; 1.1072x over previous
"""Trainium2 Bass kernel for nn_CrossAttention_82429012345074.

8-head self-attention, B=2, N=4096, d_model=512, 8 heads x 64 dim.

Sharding: one head per NeuronCore (8 heads / 8 cores) — tensor parallel:
to_q/k/v column-parallel (each core gets its head's 64 rows of Wq/Wk/Wv),
to_out row-parallel (each core gets its head's 64 columns of Wo and emits a
partial [tok, 512] output). The unshard step sums the 8 partials + bias on
host.

Per-core device kernel (matmuls bf16, fp32 accumulation):
  xT = x pre-transposed on host            # [512f, 8192t] in 4 chunks
  qT/kT = W_dup @ xT                       # [128, 8192] (dh duplicated hi/lo)
  v     = xT.T @ Wv (natural layout)       # [8192, 64] + ones column
  per (batch, 512-query group), chunks of 2 key blocks:
     sA/sB = concurrent row-tile matmul pair (tile_position (0,0)/(64,0))
     p = exp(s * scale): alternating engines —
         ScalarE: ACT exp activation (PSUM->SBUF bf16)
         VectorE: Schraudolph bit-trick exp: int16(s*A+B) bits == bf16 exp
     o[65, q] += [v|1].T @ p               # accumulate; row 64 = denom
     epilogue (denT transpose, recip, out-proj, scaled store) interleaved
     into the next group's chunk loop.
"""

import sys

sys.path.insert(0, "/opt/trn_rl_repo")

import numpy as np
import ml_dtypes

B, N, D, H, DH = 2, 4096, 512, 8, 64
TOK = B * N            # 8192
NQ = 512               # query-group width
NCH = D // 128         # 4 feature chunks of x
NJB = N // 128         # 32 key blocks per batch
NTB = TOK // 128       # 64 token blocks
CH = 2                 # key blocks per exp chunk
NCK = NJB // CH        # 16 chunks per group
SCALE = DH ** -0.5
# Schraudolph exp: fp8e4m3 bits of exp(s*SCALE) ~= int8(s*EXPA + EXPB)
EXPA = float(SCALE * 8 / np.log(2.0))
EXPB = float(7 * 8 - 0.45)
DVE_CHUNKS = frozenset(range(1, 16, 2))  # exp chunks on VectorE (odd ci)
VW = 80                # vP row pitch (fp8, %16==0 for DoubleRow ldweights)
PVD = 3                # PV deferral depth in chunks


def build_bass():
    from contextlib import ExitStack

    import concourse.bass as bass
    import concourse.mybir as mybir
    import concourse.tile as tile
    from concourse import bacc

    f32 = mybir.dt.float32
    bf16 = mybir.dt.bfloat16
    fp8 = mybir.dt.float8e4
    i8 = mybir.dt.int8
    EXP = mybir.ActivationFunctionType.Exp
    MUL = mybir.AluOpType.mult
    ADD = mybir.AluOpType.add
    DR = mybir.MatmulPerfMode.DoubleRow

    nc = bacc.Bacc("TRN2", target_bir_lowering=False, num_devices=8)
    x_d = nc.dram_tensor("x", [NCH, 128, TOK], bf16, kind="ExternalInput")
    wq_d = nc.dram_tensor("wq", [128, NCH, 128], bf16, kind="ExternalInput")
    wk_d = nc.dram_tensor("wk", [128, NCH, 128], bf16, kind="ExternalInput")
    wv_d = nc.dram_tensor("wv", [128, NCH, DH], bf16, kind="ExternalInput")
    wo_d = nc.dram_tensor("wo", [DH, D], bf16, kind="ExternalInput")
    out_d = nc.dram_tensor("out", [TOK, D], f32, kind="ExternalOutput")

    with tile.TileContext(nc) as tc, ExitStack() as ctx:
        const = ctx.enter_context(tc.tile_pool(name="const", bufs=1))
        sb_p = ctx.enter_context(tc.tile_pool(name="sb_p", bufs=4))
        sb_io = ctx.enter_context(tc.tile_pool(name="sb_io", bufs=3))
        ps_s = ctx.enter_context(tc.tile_pool(name="ps_s", bufs=3, space="PSUM"))
        ps_sm = ctx.enter_context(tc.tile_pool(name="ps_sm", bufs=1, space="PSUM"))

        # Long-lived SBUF tensors
        xT = const.tile([128, NCH, TOK], bf16, name="xT")
        qT = const.tile([128, TOK], bf16, name="qT")       # rows 64:128 dup of 0:64
        kT = const.tile([128, TOK], bf16, name="kT")
        vP = const.tile([128, NTB, VW], fp8, name="vP")
        oN = const.tile([DH + 1, TOK], bf16, name="oN")    # row DH = softmax denom
        wq = const.tile([128, NCH, 128], bf16, name="wq")
        wk = const.tile([128, NCH, 128], bf16, name="wk")
        wv = const.tile([128, NCH, DH], bf16, name="wv")
        wo = const.tile([64, D], bf16, name="wo")

        nc.vector.memset(vP[:, :, DH : DH + 1], 1.0)
        ones128 = const.tile([128, 1], bf16, name="ones128")
        nc.vector.memset(ones128, 1.0)

        nc.sync.dma_start(out=wq, in_=wq_d[:])
        nc.sync.dma_start(out=wk, in_=wk_d[:])
        nc.sync.dma_start(out=wv, in_=wv_d[:])
        nc.sync.dma_start(out=wo, in_=wo_d[:])

        # Phase 0: x slab-ordered so early projections can start while the
        # rest of x streams in.
        SLAB = 1024
        for tt in range(TOK // SLAB):
            t0 = tt * SLAB
            for c in range(NCH):
                nc.sync.dma_start(
                    out=xT[:, c, t0 : t0 + SLAB],
                    in_=x_d[c, :, t0 : t0 + SLAB],
                )

        # Phase 1: projections
        for g in range(TOK // NQ):
            t0 = g * NQ
            qp = ps_s.tile([128, NQ], f32, tag="s", name="qp")
            for c in range(NCH):
                nc.tensor.matmul(
                    qp, lhsT=wq[:, c, :], rhs=xT[:, c, t0 : t0 + NQ],
                    start=(c == 0), stop=(c == NCH - 1),
                )
            nc.scalar.copy(out=qT[:, t0 : t0 + NQ], in_=qp)

            kp = ps_s.tile([128, NQ], f32, tag="s", name="kp")
            for c in range(NCH):
                nc.tensor.matmul(
                    kp, lhsT=wk[:, c, :], rhs=xT[:, c, t0 : t0 + NQ],
                    start=(c == 0), stop=(c == NCH - 1),
                )
            nc.scalar.copy(out=kT[:, t0 : t0 + NQ], in_=kp)

            for t in range(NQ // 128):
                tb = g * (NQ // 128) + t
                vp = ps_s.tile([128, DH], f32, tag="s", name="vp")
                for c in range(NCH):
                    nc.tensor.matmul(
                        vp, lhsT=xT[:, c, tb * 128 : tb * 128 + 128], rhs=wv[:, c, :],
                        start=(c == 0), stop=(c == NCH - 1),
                    )
                nc.vector.tensor_copy(out=vP[:, tb, 0:DH], in_=vp)

        # Phase 2+3: attention + output projection.
        def emit_denT(pq0):
            denT = ps_sm.tile([128, 4], f32, tag="o", name="denT")
            for t in range(NQ // 128):
                nc.tensor.matmul(
                    denT[:, t : t + 1],
                    lhsT=oN[DH : DH + 1, pq0 + t * 128 : pq0 + (t + 1) * 128],
                    rhs=ones128[DH : DH + 1, :],
                    start=True, stop=True,
                )
            recT = sb_io.tile([128, 4], f32, tag="rec", bufs=2, name="recT")
            nc.vector.reciprocal(recT, denT)
            return recT

        fp_queue = {}

        def emit_fp(pq0, t):
            tt0 = pq0 + t * 128
            fp = ps_sm.tile([128, D], f32, tag="o", name="fp")
            nc.tensor.matmul(
                fp, lhsT=oN[0:DH, tt0 : tt0 + 128], rhs=wo, start=True, stop=True
            )
            fp_queue[t] = fp

        def emit_ob(pq0, t, recT, on_act=False):
            tt0 = pq0 + t * 128
            ob = sb_io.tile([128, D], f32, tag="ob", name="ob")
            if on_act:
                nc.scalar.mul(ob, fp_queue.pop(t), recT[:, t : t + 1])
            else:
                nc.vector.tensor_scalar_mul(
                    ob, in0=fp_queue.pop(t), scalar1=recT[:, t : t + 1]
                )
            nc.sync.dma_start(out=out_d[tt0 : tt0 + 128, :], in_=ob)

        pending = None  # q0 of the previous group
        recT = None
        for b in range(B):
            for qg in range(N // NQ):
                q0 = b * N + qg * NQ
                o = ps_sm.tile([DH + 1, NQ], f32, tag="oo", name="o")
                # PV deferred PVD chunks so each exp overlaps ~2 chunks of
                # PE work before its PV drains it.
                pv_queue = []  # (p tile, chunk index) awaiting emission

                def flush_pv(pv):
                    p, pci = pv
                    jbg = b * NJB + pci * CH
                    nc.tensor.matmul(
                        o, lhsT=vP[:, jbg : jbg + CH, 0 : DH + 1],
                        rhs=p[:, 0:CH, :],
                        start=(pci == 0), stop=(pci == NCK - 1),
                        perf_mode=DR,
                    )

                for ci in range(NCK):
                    jb = ci * CH
                    s = ps_s.tile([128, CH, NQ], f32, tag="s", name="s")
                    jA = b * N + jb * 128
                    jB = b * N + (jb + 1) * 128
                    nc.tensor.matmul(
                        s[:, 0, :], lhsT=kT[0:64, jA : jA + 128],
                        rhs=qT[0:64, q0 : q0 + NQ], start=True, stop=True,
                    )
                    nc.tensor.matmul(
                        s[:, 1, :], lhsT=kT[64:128, jB : jB + 128],
                        rhs=qT[64:128, q0 : q0 + NQ], start=True, stop=True,
                    )
                    p = sb_p.tile([128, CH, NQ], fp8, name="p")
                    if ci in DVE_CHUNKS:
                        nc.vector.tensor_scalar(
                            out=p.bitcast(i8), in0=s[:, 0:CH, :],
                            scalar1=EXPA, scalar2=EXPB, op0=MUL, op1=ADD,
                        )
                    else:
                        nc.scalar.activation(
                            out=p[:, 0:CH, :], in_=s[:, 0:CH, :], func=EXP, scale=SCALE
                        )
                    if len(pv_queue) >= PVD:
                        flush_pv(pv_queue.pop(0))
                    pv_queue.append((p, ci))
                    # previous group's epilogue, spread across this loop
                    if pending is not None:
                        if ci == 1:
                            recT = emit_denT(pending)
                        elif ci in (3, 6, 9, 12):
                            emit_fp(pending, (ci - 3) // 3)
                        elif ci in (4, 7, 10, 13):
                            t = (ci - 4) // 3
                            emit_ob(pending, t, recT, on_act=(t < 3))
                for pv in pv_queue:
                    flush_pv(pv)

                nc.vector.tensor_copy(out=oN[:, q0 : q0 + NQ], in_=o)
                pending = q0

        recT = emit_denT(pending)
        for t in range(NQ // 128):
            emit_fp(pending, t)
            emit_ob(pending, t, recT)

    nc.compile()
    return nc


def make_in_maps(x, Wq, Wk, Wv, Wo):
    bf16 = ml_dtypes.bfloat16
    x_bf = np.ascontiguousarray(
        x.reshape(TOK, D).T.reshape(NCH, 128, TOK)
    ).astype(bf16)
    in_maps = []
    for h in range(H):
        sl = slice(h * DH, (h + 1) * DH)
        # [512, 64] -> [NCH, 128, 64] -> dup dh -> [128, NCH, 128]
        wqc = Wq[sl, :].T.reshape(NCH, 128, DH)
        wkc = Wk[sl, :].T.reshape(NCH, 128, DH)
        wvc = Wv[sl, :].T.reshape(NCH, 128, DH)
        in_maps.append(
            {
                "x": x_bf,
                "wq": np.ascontiguousarray(
                    np.concatenate([wqc, wqc], axis=2).transpose(1, 0, 2)
                ).astype(bf16),
                "wk": np.ascontiguousarray(
                    np.concatenate([wkc, wkc], axis=2).transpose(1, 0, 2)
                ).astype(bf16),
                "wv": np.ascontiguousarray(wvc.transpose(1, 0, 2)).astype(bf16),
                "wo": np.ascontiguousarray(Wo[:, sl].T).astype(bf16),
            }
        )
    return in_maps


def _install_ntff_shim():
    """The axon boot skips registering the NTFF profile hook when the image's
    antenv lacks axon_hooks; register an equivalent shim so trace=True works."""
    import types

    if "antenv.axon_hooks" in sys.modules:
        return
    try:
        from trn_agent_boot.trn_boot import _ntff_profile_via_ctypes

        hook = _ntff_profile_via_ctypes("/opt/axon/libaxon_pjrt.so")
    except Exception:
        hook = None
    mod = types.ModuleType("antenv.axon_hooks")
    mod.get_axon_ntff_profile_hook = lambda: hook
    sys.modules["antenv.axon_hooks"] = mod


def run(x, Wq, Wk, Wv, Wo, bo, trace=False):
    from concourse.bass_utils import run_bass_kernel_spmd

    if trace:
        _install_ntff_shim()

    nc = build_bass()
    in_maps = make_in_maps(x, Wq, Wk, Wv, Wo)
    res = run_bass_kernel_spmd(nc, in_maps, core_ids=list(range(H)), trace=trace)
    acc = np.zeros((TOK, D), dtype=np.float32)
    for r in res.results:
        acc += r["out"]
    acc += np.asarray(bo, dtype=np.float32)[None, :]
    return acc.reshape(B, N, D), res


def kernel(x, Wq, Wk, Wv, Wo, bo):
    out, _ = run(
        np.asarray(x, dtype=np.float32),
        np.asarray(Wq, dtype=np.float32),
        np.asarray(Wk, dtype=np.float32),
        np.asarray(Wv, dtype=np.float32),
        np.asarray(Wo, dtype=np.float32),
        np.asarray(bo, dtype=np.float32),
    )
    return out


if __name__ == "__main__":
    import reference

    inputs = reference.setup_inputs()
    inputs = {k: np.asarray(v) for k, v in inputs.items()}
    expected = np.asarray(reference.reference(**inputs))
    out = kernel(**inputs)
    rel = np.linalg.norm(out - expected) / np.linalg.norm(expected)
    print(f"Relative error: {rel:.3e}")


# revision 16
# speedup vs baseline: 1.1186x; 1.0103x over previous
"""Trainium2 Bass kernel for nn_CrossAttention_82429012345074.

8-head self-attention, B=2, N=4096, d_model=512, 8 heads x 64 dim.

Sharding: one head per NeuronCore (8 heads / 8 cores) — tensor parallel:
to_q/k/v column-parallel (each core gets its head's 64 rows of Wq/Wk/Wv),
to_out row-parallel (each core gets its head's 64 columns of Wo and emits a
partial [tok, 512] output). The unshard step sums the 8 partials + bias on
host.

Per-core device kernel (matmuls bf16, fp32 accumulation):
  xT = x pre-transposed on host            # [512f, 8192t] in 4 chunks
  qT/kT = W_dup @ xT                       # [128, 8192] (dh duplicated hi/lo)
  v     = xT.T @ Wv (natural layout)       # [8192, 64] + ones column
  per (batch, 512-query group), chunks of 2 key blocks:
     sA/sB = concurrent row-tile matmul pair (tile_position (0,0)/(64,0))
     p = exp(s * scale): alternating engines —
         ScalarE: ACT exp activation (PSUM->SBUF bf16)
         VectorE: Schraudolph bit-trick exp: int16(s*A+B) bits == bf16 exp
     o[65, q] += [v|1].T @ p               # accumulate; row 64 = denom
     epilogue (denT transpose, recip, out-proj, scaled store) interleaved
     into the next group's chunk loop.
"""

import sys

sys.path.insert(0, "/opt/trn_rl_repo")

import numpy as np
import ml_dtypes

B, N, D, H, DH = 2, 4096, 512, 8, 64
TOK = B * N            # 8192
NQ = 512               # query-group width
NCH = D // 128         # 4 feature chunks of x
NJB = N // 128         # 32 key blocks per batch
NTB = TOK // 128       # 64 token blocks
CH = 2                 # key blocks per exp chunk
NCK = NJB // CH        # 16 chunks per group
SCALE = DH ** -0.5
# Schraudolph exp: fp8e4m3 bits of exp(s*SCALE) ~= int8(s*EXPA + EXPB)
EXPA = float(SCALE * 8 / np.log(2.0))
EXPB = float(7 * 8 - 0.45)
DVE_CHUNKS = frozenset(range(1, 16, 2))  # exp chunks on VectorE (odd ci)
VW = 80                # vP row pitch (fp8, %16==0 for DoubleRow ldweights)
PVD = 3                # PV deferral depth in chunks


def build_bass():
    from contextlib import ExitStack

    import concourse.bass as bass
    import concourse.mybir as mybir
    import concourse.tile as tile
    from concourse import bacc

    f32 = mybir.dt.float32
    bf16 = mybir.dt.bfloat16
    fp8 = mybir.dt.float8e4
    i8 = mybir.dt.int8
    EXP = mybir.ActivationFunctionType.Exp
    MUL = mybir.AluOpType.mult
    ADD = mybir.AluOpType.add
    DR = mybir.MatmulPerfMode.DoubleRow

    nc = bacc.Bacc("TRN2", target_bir_lowering=False, num_devices=8)
    x_d = nc.dram_tensor("x", [NCH, 128, TOK], bf16, kind="ExternalInput")
    wq_d = nc.dram_tensor("wq", [128, NCH, 128], bf16, kind="ExternalInput")
    wk_d = nc.dram_tensor("wk", [128, NCH, 128], bf16, kind="ExternalInput")
    wv_d = nc.dram_tensor("wv", [128, NCH, DH], bf16, kind="ExternalInput")
    wo_d = nc.dram_tensor("wo", [DH, D], bf16, kind="ExternalInput")
    out_d = nc.dram_tensor("out", [TOK, D], f32, kind="ExternalOutput")

    with tile.TileContext(nc) as tc, ExitStack() as ctx:
        const = ctx.enter_context(tc.tile_pool(name="const", bufs=1))
        sb_p = ctx.enter_context(tc.tile_pool(name="sb_p", bufs=4))
        sb_io = ctx.enter_context(tc.tile_pool(name="sb_io", bufs=3))
        ps_s = ctx.enter_context(tc.tile_pool(name="ps_s", bufs=3, space="PSUM"))
        ps_sm = ctx.enter_context(tc.tile_pool(name="ps_sm", bufs=1, space="PSUM"))

        # Long-lived SBUF tensors
        xT = const.tile([128, NCH, TOK], bf16, name="xT")
        qT = const.tile([128, TOK], bf16, name="qT")       # rows 64:128 dup of 0:64
        kT = const.tile([128, TOK], bf16, name="kT")
        vP = const.tile([128, NTB, VW], fp8, name="vP")
        oN = const.tile([DH + 1, TOK], bf16, name="oN")    # row DH = softmax denom
        wq = const.tile([128, NCH, 128], bf16, name="wq")
        wk = const.tile([128, NCH, 128], bf16, name="wk")
        wv = const.tile([128, NCH, DH], bf16, name="wv")
        wo = const.tile([64, D], bf16, name="wo")

        nc.vector.memset(vP[:, :, DH : DH + 1], 1.0)
        ones128 = const.tile([128, 1], bf16, name="ones128")
        nc.vector.memset(ones128, 1.0)

        nc.scalar.dma_start(out=wq, in_=wq_d[:])
        nc.scalar.dma_start(out=wk, in_=wk_d[:])
        nc.scalar.dma_start(out=wv, in_=wv_d[:])
        nc.scalar.dma_start(out=wo, in_=wo_d[:])

        # Phase 0: x slab-ordered so early projections can start while the
        # rest of x streams in. First slab split across both DMA-issuing
        # engine queues so it lands fast.
        SLAB = 1024
        for c in range(NCH):
            h = SLAB // 2
            nc.sync.dma_start(out=xT[:, c, 0:h], in_=x_d[c, :, 0:h])
            nc.scalar.dma_start(out=xT[:, c, h:SLAB], in_=x_d[c, :, h:SLAB])
        for tt in range(1, TOK // SLAB):
            t0 = tt * SLAB
            for c in range(NCH):
                nc.sync.dma_start(
                    out=xT[:, c, t0 : t0 + SLAB],
                    in_=x_d[c, :, t0 : t0 + SLAB],
                )

        # Phase 1: projections. Groups 0-7 (batch 0) emitted up front; groups
        # 8-15 (batch 1) are injected piecewise into batch-0 attention chunks
        # to fill PE/engine slack there.
        def emit_proj_piece(g, piece):
            t0 = g * NQ
            if piece == 0:
                qp = ps_s.tile([128, NQ], f32, tag="s", name="qp")
                for c in range(NCH):
                    nc.tensor.matmul(
                        qp, lhsT=wq[:, c, :], rhs=xT[:, c, t0 : t0 + NQ],
                        start=(c == 0), stop=(c == NCH - 1),
                    )
                nc.scalar.copy(out=qT[:, t0 : t0 + NQ], in_=qp)
            elif piece == 1:
                kp = ps_s.tile([128, NQ], f32, tag="s", name="kp")
                for c in range(NCH):
                    nc.tensor.matmul(
                        kp, lhsT=wk[:, c, :], rhs=xT[:, c, t0 : t0 + NQ],
                        start=(c == 0), stop=(c == NCH - 1),
                    )
                nc.scalar.copy(out=kT[:, t0 : t0 + NQ], in_=kp)
            else:
                tb = g * (NQ // 128) + (piece - 2)
                vp = ps_s.tile([128, DH], f32, tag="s", name="vp")
                for c in range(NCH):
                    nc.tensor.matmul(
                        vp, lhsT=xT[:, c, tb * 128 : tb * 128 + 128], rhs=wv[:, c, :],
                        start=(c == 0), stop=(c == NCH - 1),
                    )
                nc.vector.tensor_copy(out=vP[:, tb, 0:DH], in_=vp)

        for g in range(8):
            for piece in range(6):
                emit_proj_piece(g, piece)

        # Phase 2+3: attention + output projection.
        def emit_denT(pq0):
            denT = ps_sm.tile([128, 4], f32, tag="o", name="denT")
            for t in range(NQ // 128):
                nc.tensor.matmul(
                    denT[:, t : t + 1],
                    lhsT=oN[DH : DH + 1, pq0 + t * 128 : pq0 + (t + 1) * 128],
                    rhs=ones128[DH : DH + 1, :],
                    start=True, stop=True,
                )
            recT = sb_io.tile([128, 4], f32, tag="rec", bufs=2, name="recT")
            nc.vector.reciprocal(recT, denT)
            return recT

        fp_queue = {}

        def emit_fp(pq0, t):
            tt0 = pq0 + t * 128
            fp = ps_sm.tile([128, D], f32, tag="o", name="fp")
            nc.tensor.matmul(
                fp, lhsT=oN[0:DH, tt0 : tt0 + 128], rhs=wo, start=True, stop=True
            )
            fp_queue[t] = fp

        def emit_ob(pq0, t, recT, on_act=False):
            tt0 = pq0 + t * 128
            ob = sb_io.tile([128, D], f32, tag="ob", name="ob")
            if on_act:
                nc.scalar.mul(ob, fp_queue.pop(t), recT[:, t : t + 1])
            else:
                nc.vector.tensor_scalar_mul(
                    ob, in0=fp_queue.pop(t), scalar1=recT[:, t : t + 1]
                )
            nc.sync.dma_start(out=out_d[tt0 : tt0 + 128, :], in_=ob)

        pending = None  # q0 of the previous group
        recT = None
        for b in range(B):
            for qg in range(N // NQ):
                q0 = b * N + qg * NQ
                o = ps_sm.tile([DH + 1, NQ], f32, tag="oo", name="o")
                # PV deferred PVD chunks so each exp overlaps ~2 chunks of
                # PE work before its PV drains it.
                pv_queue = []  # (p tile, chunk index) awaiting emission

                def flush_pv(pv):
                    p, pci = pv
                    jbg = b * NJB + pci * CH
                    nc.tensor.matmul(
                        o, lhsT=vP[:, jbg : jbg + CH, 0 : DH + 1],
                        rhs=p[:, 0:CH, :],
                        start=(pci == 0), stop=(pci == NCK - 1),
                        perf_mode=DR,
                    )

                for ci in range(NCK):
                    jb = ci * CH
                    s = ps_s.tile([128, CH, NQ], f32, tag="s", name="s")
                    jA = b * N + jb * 128
                    jB = b * N + (jb + 1) * 128
                    nc.tensor.matmul(
                        s[:, 0, :], lhsT=kT[0:64, jA : jA + 128],
                        rhs=qT[0:64, q0 : q0 + NQ], start=True, stop=True,
                    )
                    nc.tensor.matmul(
                        s[:, 1, :], lhsT=kT[64:128, jB : jB + 128],
                        rhs=qT[64:128, q0 : q0 + NQ], start=True, stop=True,
                    )
                    p = sb_p.tile([128, CH, NQ], fp8, name="p")
                    if ci in DVE_CHUNKS:
                        nc.vector.tensor_scalar(
                            out=p.bitcast(i8), in0=s[:, 0:CH, :],
                            scalar1=EXPA, scalar2=EXPB, op0=MUL, op1=ADD,
                        )
                    else:
                        nc.scalar.activation(
                            out=p[:, 0:CH, :], in_=s[:, 0:CH, :], func=EXP, scale=SCALE
                        )
                    if len(pv_queue) >= PVD:
                        flush_pv(pv_queue.pop(0))
                    pv_queue.append((p, ci))
                    # batch-1 projections injected into batch-0 attention
                    if b == 0 and ci in (2, 5, 7, 9, 11, 14):
                        emit_proj_piece(8 + qg, (2, 5, 7, 9, 11, 14).index(ci))
                    # previous group's epilogue, spread across this loop
                    if pending is not None:
                        if ci == 1:
                            recT = emit_denT(pending)
                        elif ci in (3, 6, 9, 12):
                            emit_fp(pending, (ci - 3) // 3)
                        elif ci in (4, 7, 10, 13):
                            t = (ci - 4) // 3
                            emit_ob(pending, t, recT, on_act=(t < 3))
                for pv in pv_queue:
                    flush_pv(pv)

                nc.vector.tensor_copy(out=oN[:, q0 : q0 + NQ], in_=o)
                pending = q0

        recT = emit_denT(pending)
        for t in range(NQ // 128):
            emit_fp(pending, t)
            emit_ob(pending, t, recT)

    nc.compile()
    return nc


def make_in_maps(x, Wq, Wk, Wv, Wo):
    bf16 = ml_dtypes.bfloat16
    x_bf = np.ascontiguousarray(
        x.reshape(TOK, D).T.reshape(NCH, 128, TOK)
    ).astype(bf16)
    in_maps = []
    for h in range(H):
        sl = slice(h * DH, (h + 1) * DH)
        # [512, 64] -> [NCH, 128, 64] -> dup dh -> [128, NCH, 128]
        wqc = Wq[sl, :].T.reshape(NCH, 128, DH)
        wkc = Wk[sl, :].T.reshape(NCH, 128, DH)
        wvc = Wv[sl, :].T.reshape(NCH, 128, DH)
        in_maps.append(
            {
                "x": x_bf,
                "wq": np.ascontiguousarray(
                    np.concatenate([wqc, wqc], axis=2).transpose(1, 0, 2)
                ).astype(bf16),
                "wk": np.ascontiguousarray(
                    np.concatenate([wkc, wkc], axis=2).transpose(1, 0, 2)
                ).astype(bf16),
                "wv": np.ascontiguousarray(wvc.transpose(1, 0, 2)).astype(bf16),
                "wo": np.ascontiguousarray(Wo[:, sl].T).astype(bf16),
            }
        )
    return in_maps


def _install_ntff_shim():
    """The axon boot skips registering the NTFF profile hook when the image's
    antenv lacks axon_hooks; register an equivalent shim so trace=True works."""
    import types

    if "antenv.axon_hooks" in sys.modules:
        return
    try:
        from trn_agent_boot.trn_boot import _ntff_profile_via_ctypes

        hook = _ntff_profile_via_ctypes("/opt/axon/libaxon_pjrt.so")
    except Exception:
        hook = None
    mod = types.ModuleType("antenv.axon_hooks")
    mod.get_axon_ntff_profile_hook = lambda: hook
    sys.modules["antenv.axon_hooks"] = mod


def run(x, Wq, Wk, Wv, Wo, bo, trace=False):
    from concourse.bass_utils import run_bass_kernel_spmd

    if trace:
        _install_ntff_shim()

    nc = build_bass()
    in_maps = make_in_maps(x, Wq, Wk, Wv, Wo)
    res = run_bass_kernel_spmd(nc, in_maps, core_ids=list(range(H)), trace=trace)
    acc = np.zeros((TOK, D), dtype=np.float32)
    for r in res.results:
        acc += r["out"]
    acc += np.asarray(bo, dtype=np.float32)[None, :]
    return acc.reshape(B, N, D), res


def kernel(x, Wq, Wk, Wv, Wo, bo):
    out, _ = run(
        np.asarray(x, dtype=np.float32),
        np.asarray(Wq, dtype=np.float32),
        np.asarray(Wk, dtype=np.float32),
        np.asarray(Wv, dtype=np.float32),
        np.asarray(Wo, dtype=np.float32),
        np.asarray(bo, dtype=np.float32),
    )
    return out


if __name__ == "__main__":
    import reference

    inputs = reference.setup_inputs()
    inputs = {k: np.asarray(v) for k, v in inputs.items()}
    expected = np.asarray(reference.reference(**inputs))
    out = kernel(**inputs)
    rel = np.linalg.norm(out - expected) / np.linalg.norm(expected)
    print(f"Relative error: {rel:.3e}")
